# revision 1
# baseline (speedup 1.0000x reference)
"""Type-2 NUFFT (image -> non-uniform k-space) on 8 Trainium2 NeuronCores.

kspace[b,m] = sum_{x,y} image[b,x,y] * exp(-i*(kx_m*(x-128) + ky_m*(y-128)))

Quarter-fold decomposition with half-integer centering: write
x-128 = v - 1/2 with v = x - 127.5 in +-{0.5, ..., 127.5}, and likewise
y-128 = u - 1/2.  Then

  kspace[b,m] = e^{i(kx+ky)/2} * sum_{v,u} img * e^{-i(kx v + ky u)}

and the inner sum folds EXACTLY into 128x128 quadrant images (cos is even,
sin is odd in both v and u):

  inner = R - i*N
  R[m] = sum_w C~E[m,w]*cosY[m,w] - S~O[m,w]*sinY[m,w]
  N[m] = sum_w C~O[m,w]*sinY[m,w] + S~E[m,w]*cosY[m,w]
  C~E = cosX^T @ imgEE   C~O = cosX^T @ imgEO      (per batch)
  S~E = sinX^T @ imgOE   S~O = sinX^T @ imgOO

Work split: the host (numpy) computes the trig tables (cosX/sinX per m-tile,
cosY/sinY per m-tile) and the folded quadrant images in bf16, packs them
into one consumption-ordered blob, and applies the final e^{i(kx+ky)/2}
rotation.  The device does the O(M*N^2) work only: per (batch, m-tile) two
bf16 matmuls into PSUM and two fused DVE multiply+row-reduce ops that
accumulate straight into the output column.
"""

import sys

if '/opt/trn_rl_repo' not in sys.path:
    sys.path.insert(0, '/opt/trn_rl_repo')

import numpy as np
import ml_dtypes

B, NX, NY, M, NCORES = 2, 256, 256, 16384, 8
ML = M // NCORES            # 2048 m-points per core
NT = ML // 128              # 16 m-tiles per core

# blob layout (bf16, per partition-col), ordered by first consumption —
# the head is split so the first matmul's inputs land in a tiny chunk:
#   [imgC b0 (256) | cx0 (128) | imgS b0 (256) | sx0 (128) | w0 (256) |
#    img b1 (512) | t1..t15 tables]
# where a t>=1 table block is [cx(128) | sx(128) | w=cosY|sinY (256)].
TSTRIDE = 512
BLOB_COLS = 1536 + (NT - 1) * 512


def _cxcol(t):
    return 256 if t == 0 else 1536 + (t - 1) * TSTRIDE


def _sxcol(t):
    return 640 if t == 0 else 1536 + (t - 1) * TSTRIDE + 128


def _wcol(t):
    return 768 if t == 0 else 1536 + (t - 1) * TSTRIDE + 256


def _imgccol(b):
    return 0 if b == 0 else 1024


def _imgscol(b):
    return 384 if b == 0 else 1280

_CACHE = {}


_C = {1, 7, 13, 18, 23, 27, 29}
_B = {3, 9, 15, 20, 25}
DEFAULT_PATTERN = ''.join('C' if i in _C else 'B' if i in _B else 'A'
                          for i in range(32))


def _build(pattern=DEFAULT_PATTERN, psum_bufs=6, work_bufs=10, nchunks=None,
           out_every=8, out_marks=(1, 3, 7, 11, 13, 15), warm=0):
    import concourse.bacc as bacc
    import concourse.bass as bass
    import concourse.mybir as mybir
    from concourse.tile import TileContext

    A = mybir.AluOpType
    f32 = mybir.dt.float32
    bf16 = mybir.dt.bfloat16

    nc = bacc.Bacc("TRN2", target_bir_lowering=False, debug=False)

    blob = nc.dram_tensor("blob", [128, BLOB_COLS], bf16, kind="ExternalInput")
    out = nc.dram_tensor("out", [128, 4 * NT], f32, kind="ExternalOutput")

    def seg2(tile_ap, start, seg_stride):
        """[128, 2, 128] view: two 128-wide segments at start, start+stride."""
        t_ = tile_ap.tensor
        row = tile_ap.ap[0][0]
        return bass.AP(t_, tile_ap.offset + start,
                       [[row, 128], [seg_stride, 2], [1, 128]])

    # DMA chunks over the blob, in consumption order (first small, for a
    # fast pipeline start)
    if nchunks is None:
        bounds = [0, 768, 1536, _cxcol(2), _cxcol(3), _cxcol(5), _cxcol(8),
                  _cxcol(11), _cxcol(14), _cxcol(15), BLOB_COLS]
    else:
        bounds = nchunks

    with TileContext(nc) as tc:
        with tc.tile_pool(name="const", bufs=1) as cpool, \
             tc.tile_pool(name="work", bufs=work_bufs) as wpool, \
             tc.tile_pool(name="ps", bufs=psum_bufs, space="PSUM") as ps:

            bsb = cpool.tile([128, BLOB_COLS], bf16, name="blob")
            out_sb = cpool.tile([128, 4 * NT], f32)

            for i in range(len(bounds) - 1):
                cs = slice(bounds[i], bounds[i + 1])
                nc.sync.dma_start(bsb[:, cs], blob[:, cs])

            if warm:
                # keep the PE continuously busy during the DMA startup so the
                # p-state ramp reaches full clock before the real matmuls
                wsrc = cpool.tile([128, 512], bf16, name="warmsrc")
                nc.gpsimd.memset(wsrc[:, :], 1.0)
                wab = ps.tile([128, 512], f32, tag="warm", bufs=1)
                for _ in range(warm):
                    nc.tensor.matmul(wab[:, :], wsrc[:, 0:128], wsrc[:, :],
                                     start=True, stop=True)

            # per-(t,b) stage-2 path: A = DVE fused multiply+reduce from PSUM;
            # B = Act evicts PSUM->SBUF, Pool multiplies, DVE reduces (2x);
            # C = like B but Act reduces.  Balances DVE/Act/Pool busy time.
            PATTERN = pattern
            F = mybir.ActivationFunctionType

            for t in range(NT):
                cxc, sxc, wc = _cxcol(t), _sxcol(t), _wcol(t)
                for b in range(B):
                    ic, isc = _imgccol(b), _imgscol(b)
                    ab = ps.tile([128, 512], f32, tag="ab")
                    # ab = [C~E | C~O | S~E | -S~O]
                    nc.tensor.matmul(ab[:, 0:256],
                                     bsb[:, cxc:cxc + 128],
                                     bsb[:, ic:ic + 256],
                                     start=True, stop=True)
                    nc.tensor.matmul(ab[:, 256:512],
                                     bsb[:, sxc:sxc + 128],
                                     bsb[:, isc:isc + 256],
                                     start=True, stop=True)
                    col = t * 4 + b * 2
                    path = PATTERN[t * 2 + b]
                    # R = sum(C~E*cosY) + sum(-S~O*sinY)   -> col
                    # N = sum(C~O*sinY) + sum(S~E*cosY)    -> col+1
                    if path == 'A':
                        scr = wpool.tile([128, 256], f32, tag="scr")
                        scr2 = wpool.tile([128, 256], f32, tag="scr2")
                        nc.vector.scalar_tensor_tensor(
                            seg2(scr[:, :], 0, 128),
                            seg2(ab[:, :], 0, 384), 1.0,
                            seg2(bsb[:, :], wc, 128),
                            op0=A.mult, op1=A.mult,
                            accum_out=out_sb[:, col:col + 1])
                        nc.vector.scalar_tensor_tensor(
                            seg2(scr2[:, :], 0, 128),
                            seg2(ab[:, :], 128, 128), 1.0,
                            seg2(bsb[:, :], wc + 128, -128),
                            op0=A.mult, op1=A.mult,
                            accum_out=out_sb[:, col + 1:col + 2])
                    elif path == 'Q':
                        # Act evicts as bf16; DVE does 2x-mode products and
                        # 4x-mode reduces entirely from SBUF (no Pool)
                        cq = wpool.tile([128, 512], bf16, tag="cq")
                        nc.scalar.copy(cq[:, :], ab[:, :])
                        for comp, (s0, st, w0, ws) in enumerate(
                                [(0, 384, wc, 128), (128, 128, wc + 128, -128)]):
                            q1 = wpool.tile([128, 256], bf16, tag=f"q{comp}")
                            nc.vector.tensor_tensor(
                                seg2(q1[:, :], 0, 128),
                                seg2(cq[:, :], s0, st),
                                seg2(bsb[:, :], w0, ws), op=A.mult)
                            dq = wpool.tile([128, 256], bf16, tag=f"dq{comp}")
                            nc.vector.tensor_scalar(
                                dq[:, :], q1[:, :], scalar1=1.0, scalar2=0.0,
                                op0=A.mult, op1=A.add,
                                accum_out=out_sb[:, col + comp:col + comp + 1])
                    else:
                        cp = wpool.tile([128, 512], f32, tag="cp")
                        if path in 'DE':
                            # DMA engines evict PSUM->SBUF (no engine time)
                            nc.sync.dma_start(cp[:, :], ab[:, :])
                        else:
                            nc.scalar.copy(cp[:, :], ab[:, :])
                        # bf16 products let the DVE 'B'-path reduce run in
                        # 4x mode (all-bf16 packed SBUF operands)
                        pdt = bf16 if path == 'B' else f32
                        p1 = wpool.tile([128, 256], pdt, tag="p1")
                        nc.gpsimd.tensor_tensor(
                            seg2(p1[:, :], 0, 128),
                            seg2(cp[:, :], 0, 384),
                            seg2(bsb[:, :], wc, 128), op=A.mult)
                        if path in 'FG':
                            # R: reduce Pool's product (DVE 2x ts or Act);
                            # N: DVE fused stt on the SBUF copy
                            d1 = wpool.tile([128, 256], f32, tag="d1")
                            if path == 'F':
                                nc.vector.tensor_scalar(
                                    d1[:, :], p1[:, :], scalar1=1.0,
                                    scalar2=0.0, op0=A.mult, op1=A.add,
                                    accum_out=out_sb[:, col:col + 1])
                            else:
                                nc.scalar.activation(
                                    d1[:, :], p1[:, :], F.Copy,
                                    accum_out=out_sb[:, col:col + 1])
                            scr2 = wpool.tile([128, 256], f32, tag="scr2")
                            nc.vector.scalar_tensor_tensor(
                                seg2(scr2[:, :], 0, 128),
                                seg2(cp[:, :], 128, 128), 1.0,
                                seg2(bsb[:, :], wc + 128, -128),
                                op0=A.mult, op1=A.mult,
                                accum_out=out_sb[:, col + 1:col + 2])
                            continue
                        p2 = wpool.tile([128, 256], pdt, tag="p2")
                        nc.gpsimd.tensor_tensor(
                            seg2(p2[:, :], 0, 128),
                            seg2(cp[:, :], 128, 128),
                            seg2(bsb[:, :], wc + 128, -128), op=A.mult)
                        if path in 'BEP':
                            eng2 = nc.gpsimd if path == 'P' else nc.vector
                            d1 = wpool.tile([128, 256], pdt, tag="d1")
                            d2 = wpool.tile([128, 256], pdt, tag="d2")
                            eng2.tensor_scalar(
                                d1[:, :], p1[:, :], scalar1=1.0, scalar2=0.0,
                                op0=A.mult, op1=A.add,
                                accum_out=out_sb[:, col:col + 1])
                            eng2.tensor_scalar(
                                d2[:, :], p2[:, :], scalar1=1.0, scalar2=0.0,
                                op0=A.mult, op1=A.add,
                                accum_out=out_sb[:, col + 1:col + 2])
                        else:
                            d1 = wpool.tile([128, 256], f32, tag="d1")
                            d2 = wpool.tile([128, 256], f32, tag="d2")
                            nc.scalar.activation(
                                d1[:, :], p1[:, :], F.Copy,
                                accum_out=out_sb[:, col:col + 1])
                            nc.scalar.activation(
                                d2[:, :], p2[:, :], F.Copy,
                                accum_out=out_sb[:, col + 1:col + 2])
                marks = (out_marks if out_marks is not None
                         else list(range(out_every - 1, NT, out_every)))
                if t in marks:
                    prev = max([m for m in marks if m < t], default=-1)
                    qs = slice((prev + 1) * 4, (t + 1) * 4)
                    nc.sync.dma_start(out[:, qs], out_sb[:, qs])

    nc.compile()
    return nc


def _host_prep(image, trajectory):
    """Folded quadrant images + trig tables (bf16) packed per-core blobs."""
    bf = ml_dtypes.bfloat16
    kx = trajectory[0].astype(np.float32)            # [M]
    ky = trajectory[1].astype(np.float32)
    v = (np.arange(128, dtype=np.float32) + 0.5)

    cosX = np.cos(kx[None, :] * v[:, None])          # [128, M]
    sinX = np.sin(kx[None, :] * v[:, None])
    argY = ky[:, None] * v[None, :]                  # [M, 128]
    cosY = np.cos(argY)
    sinY = np.sin(argY)

    # quadrant folds (x: rows about 127.5; y: cols about 127.5)
    top = image[:, 128:256, :]
    bot = image[:, 127::-1, :]
    sumx = top + bot
    difx = top - bot
    imgEE = sumx[:, :, 128:256] + sumx[:, :, 127::-1]
    imgEO = sumx[:, :, 128:256] - sumx[:, :, 127::-1]
    imgOE = difx[:, :, 128:256] + difx[:, :, 127::-1]
    imgOOn = difx[:, :, 127::-1] - difx[:, :, 128:256]   # = -imgOO
    imgq = np.concatenate([imgEE, imgEO, imgOE, imgOOn], axis=2)  # [B,128,512]

    # per-core blob [128, BLOB_COLS]
    cx = cosX.reshape(128, NCORES, NT, 128)          # [j, c, t, p]
    sx = sinX.reshape(128, NCORES, NT, 128)
    cy = cosY.reshape(NCORES, NT, 128, 128)          # [c, t, p, w]
    sy = sinY.reshape(NCORES, NT, 128, 128)

    blobs = np.empty((NCORES, 128, BLOB_COLS), dtype=bf)
    iq0 = imgq[0].astype(bf)
    blobs[:, :, 0:256] = iq0[None, :, 0:256]       # imgC b0
    blobs[:, :, 384:640] = iq0[None, :, 256:512]   # imgS b0
    blobs[:, :, 1024:1536] = imgq[1].astype(bf)[None]
    # tables: for core c, tile t: cols [cx_t | sx_t | cy_t | sy_t]
    tbl = np.concatenate([
        cx.transpose(1, 2, 0, 3),                    # [c, t, j, p] -> cx block
        sx.transpose(1, 2, 0, 3),
        cy.transpose(0, 1, 2, 3),                    # [c, t, p, w]
        sy.transpose(0, 1, 2, 3),
    ], axis=-1).astype(bf)                            # [c, t, 128, 512]
    tblp = tbl.transpose(0, 2, 1, 3)                  # [c, 128, t, 512]
    blobs[:, :, 256:384] = tblp[:, :, 0, 0:128]       # cx0
    blobs[:, :, 640:768] = tblp[:, :, 0, 128:256]     # sx0
    blobs[:, :, 768:1024] = tblp[:, :, 0, 256:512]    # w0
    blobs[:, :, 1536:] = tblp[:, :, 1:, :].reshape(NCORES, 128,
                                                   (NT - 1) * TSTRIDE)

    phase = np.exp(1j * (kx + ky) / 2.0).astype(np.complex64)
    return blobs, phase


def kernel(image, trajectory):
    from concourse.bass_utils import run_bass_kernel_spmd

    if 'nc' not in _CACHE:
        _CACHE['nc'] = _build()
    nc = _CACHE['nc']

    image = np.ascontiguousarray(np.asarray(image, dtype=np.float32))
    trajectory = np.ascontiguousarray(np.asarray(trajectory, dtype=np.float32))
    blobs, phase = _host_prep(image, trajectory)

    in_maps = [{"blob": np.ascontiguousarray(blobs[c])} for c in range(NCORES)]

    res = run_bass_kernel_spmd(nc, in_maps, core_ids=list(range(NCORES)))

    kspace = np.empty((B, M), dtype=np.complex64)
    for c in range(NCORES):
        o = res.results[c]["out"]          # [128, 4*NT]
        o = o.reshape(128, NT, 2, 2)       # [p, t, b, (R, N)]
        for b in range(B):
            R = o[:, :, b, 0].T.reshape(ML)    # m = t*128 + p
            N = o[:, :, b, 1].T.reshape(ML)
            kspace[b, c * ML:(c + 1) * ML] = R - 1j * N
    kspace *= phase[None, :]
    return kspace



# revision 13
# speedup vs baseline: 2.4765x; 2.4765x over previous
"""Type-2 NUFFT (image -> non-uniform k-space) on 8 Trainium2 NeuronCores.

kspace[b,m] = sum_{x,y} image[b,x,y] * exp(-i*(kx_m*(x-128) + ky_m*(y-128)))

Gridding (NUFFT) formulation: with an exponential-of-semicircle kernel psi
(width J, oversampled grid S > N),

  exp(-i*k*xt) ~= (1/D(xt)) * sum_g psi(k*S/2pi - g) * exp(-i*2pi*g*xt/S)

so   kspace[b,m] ~= sum_{JxJ window} F[b,g1,g2] * w1[m]*w2[m]
with F = dense DFT of the deapodized image on the S x S grid.

Work split:
  device: the dense DFT (all the heavy FLOPs), as two matmul passes
     A[g1,y] = sum_x imgd[x,y] e1[g1,x]     (stage A, complex via 2 blocks)
     F[g1,g2] = sum_y A[g1,y] e2[g2,y]      (stage B)
   sharded over 8 cores: core = (batch b, quarter of the Hermitian-half
   g1 range [0, S/2]).  Each core outputs its F slice [49, 2*S] bf16.
  host: deapodization, trig tables, and the O(M*J^2) window interpolation
   (including Hermitian reconstruction of negative g1 rows).
"""

import sys

if '/opt/trn_rl_repo' not in sys.path:
    sys.path.insert(0, '/opt/trn_rl_repo')

import numpy as np
import ml_dtypes

B, NX, NY, M, NCORES = 2, 256, 256, 16384, 8

S = 384                  # oversampled grid (sigma = 1.5)
J = 6                    # interp kernel width (host side only)
G1H = S // 2 + 1         # Hermitian half rows: 193
NQ = 4                   # g1 quarters (cores = B * NQ)
RQ = 49                  # g1 rows per core (4*49 = 196 >= 193; tail padded)
BETA = np.pi * (J / 2.0) * (2.0 - 256.0 / S)

# blob layout (bf16 cols): [img_x0(256) | atab_x0(2*RQ) | img_x1(256) |
#                           atab_x1(2*RQ) | btab_y0(3*S) | btab_y1(3*S)]
IMG0, ATAB0 = 0, 256
IMG1, ATAB1 = 256 + 2 * RQ, 512 + 2 * RQ
BTAB0 = 512 + 4 * RQ
BTAB1 = BTAB0 + 3 * S
BLOB_COLS = BTAB1 + 3 * S

_CACHE = {}


def _es_kernel(z):
    c = J / 2.0
    out = np.zeros_like(z)
    m = np.abs(z) < c
    out[m] = np.exp(BETA * (np.sqrt(1.0 - (z[m] / c) ** 2) - 1.0))
    return out


def _deapod():
    """D(xt) = continuous FT of psi at xt/S cycles (trapezoid quadrature)."""
    c = J / 2.0
    xt = (np.arange(NX) - NX // 2).astype(np.float64)
    zq = np.linspace(-c, c, 4001)
    pz = _es_kernel(zq)
    D = np.trapezoid(pz[None, :] * np.exp(1j * 2 * np.pi * zq[None, :]
                                          * xt[:, None] / S), zq, axis=1).real
    return D


_DEAPOD = _deapod()                               # [256]
_XT = (np.arange(NX) - NX // 2).astype(np.float64)
_G2F = ((np.arange(S) + S // 2) % S - S // 2)     # col h -> g2 freq


def _tables():
    """Static device trig tables (bf16): per-quarter A tables + shared B."""
    bf = ml_dtypes.bfloat16
    atabs = []
    for q in range(NQ):
        g = np.arange(q * RQ, (q + 1) * RQ)
        g = np.minimum(g, G1H - 1)                # pad rows repeat last row
        ph = 2 * np.pi * g[None, :] * _XT[:, None] / S   # [256, RQ]
        atabs.append(np.concatenate([np.cos(ph), -np.sin(ph)],
                                    axis=1).astype(bf))  # [256, 2*RQ]
    ph2 = 2 * np.pi * _G2F[None, :] * _XT[:, None] / S   # [256, S]
    cy, sy = np.cos(ph2), np.sin(ph2)
    btab = np.concatenate([cy, sy, -sy], axis=1).astype(bf)  # [256, 3*S]
    return atabs, btab


_ATABS, _BTAB = _tables()


def _build(warm=0, nsplit=3, bchunks=2):
    import concourse.bacc as bacc
    import concourse.mybir as mybir
    from concourse.tile import TileContext

    f32 = mybir.dt.float32
    bf16 = mybir.dt.bfloat16

    nc = bacc.Bacc("TRN2", target_bir_lowering=False, debug=False)

    blob = nc.dram_tensor("blob", [128, BLOB_COLS], bf16, kind="ExternalInput")
    out = nc.dram_tensor("out", [128, 2 * S], f32, kind="ExternalOutput")

    TW = 2 * RQ      # A-tab / A cols
    with TileContext(nc) as tc:
        with tc.tile_pool(name="const", bufs=1) as cpool, \
             tc.tile_pool(name="work", bufs=4) as wpool, \
             tc.tile_pool(name="ps", bufs=1, space="PSUM") as ps:

            bsb = cpool.tile([128, BLOB_COLS], bf16, name="blob")

            # DMA chunks in consumption order; btab split for pipelining
            bounds = [0, ATAB1 + TW]
            bt_step = (BLOB_COLS - BTAB0) // bchunks
            for i in range(1, bchunks):
                bounds.append(BTAB0 + i * bt_step)
            bounds.append(BLOB_COLS)
            for i in range(len(bounds) - 1):
                cs = slice(bounds[i], bounds[i + 1])
                nc.sync.dma_start(bsb[:, cs], blob[:, cs])

            if warm:
                wsrc = cpool.tile([128, 512], bf16, name="warmsrc")
                nc.gpsimd.memset(wsrc[:, :], 1.0)
                wab = ps.tile([128, 512], f32, tag="warm", bufs=1)
                for _ in range(warm):
                    nc.tensor.matmul(wab[:, :], wsrc[:, 0:128], wsrc[:, :],
                                     start=True, stop=True)

            # stage A: A^T[y, (ArT|AiT)] per y-chunk, contract x (2 chunks)
            asb = []
            for yc in range(2):
                aps = ps.tile([128, TW], f32, tag="aps", bufs=2)
                for xc in range(2):
                    imgc = IMG0 if xc == 0 else IMG1
                    atc = ATAB0 if xc == 0 else ATAB1
                    nc.tensor.matmul(aps[:, :],
                                     bsb[:, imgc + yc * 128:imgc + yc * 128 + 128],
                                     bsb[:, atc:atc + TW],
                                     start=(xc == 0), stop=(xc == 1))
                a = wpool.tile([128, TW], bf16, tag=f"asb{yc}")
                if yc == 0:
                    nc.vector.tensor_scalar(a[:, :], aps[:, :], scalar1=1.0,
                                            scalar2=0.0, op0=mybir.AluOpType.mult,
                                            op1=mybir.AluOpType.add)
                else:
                    nc.scalar.copy(a[:, :], aps[:, :])
                asb.append(a)

            # stage B: Fr/Fi (separate PSUM banks); btab = [Cy | Sy | mSy]
            fr = ps.tile([128, S], f32, tag="fr", bufs=1)
            fi = ps.tile([128, S], f32, tag="fi", bufs=1)
            for yc in range(2):
                bt = BTAB0 if yc == 0 else BTAB1
                ar = asb[yc][:, 0:RQ]
                ai = asb[yc][:, RQ:TW]
                st, sp = (yc == 0), (yc == 1)
                # W = ArT: Fr += Ar*Cy ; Fi += Ar*(-Sy)
                nc.tensor.matmul(fr[0:RQ, :], ar, bsb[:, bt:bt + S],
                                 start=st, stop=False)
                nc.tensor.matmul(fi[0:RQ, :], ar, bsb[:, bt + 2 * S:bt + 3 * S],
                                 start=st, stop=False)
                # W = AiT: Fr += Ai*Sy ; Fi += Ai*Cy
                nc.tensor.matmul(fr[0:RQ, :], ai, bsb[:, bt + S:bt + 2 * S],
                                 start=False, stop=sp)
                nc.tensor.matmul(fi[0:RQ, :], ai, bsb[:, bt:bt + S],
                                 start=False, stop=sp)

            # evict F (f32, no rounding) DVE/Act in parallel, then DMA out
            fsb = cpool.tile([128, 2 * S], f32, name="fsb")
            nc.vector.tensor_scalar(fsb[0:RQ, 0:S], fr[0:RQ, :],
                                    scalar1=1.0, scalar2=0.0,
                                    op0=mybir.AluOpType.mult,
                                    op1=mybir.AluOpType.add)
            nc.sync.dma_start(out[0:RQ, 0:S], fsb[0:RQ, 0:S])
            nc.scalar.copy(fsb[0:RQ, S:2 * S], fi[0:RQ, :])
            nc.sync.dma_start(out[0:RQ, S:2 * S], fsb[0:RQ, S:2 * S])

    nc.compile()
    return nc


def _host_prep(image, trajectory):
    bf = ml_dtypes.bfloat16
    imgd = (image / (_DEAPOD[None, :, None] * _DEAPOD[None, None, :])
            ).astype(bf)                                   # [B, 256, 256]
    blobs = np.zeros((NCORES, 128, BLOB_COLS), dtype=bf)
    for c in range(NCORES):
        b, q = c // NQ, c % NQ
        blobs[c, :, IMG0:IMG0 + 256] = imgd[b, 0:128, :]
        blobs[c, :, IMG1:IMG1 + 256] = imgd[b, 128:256, :]
        blobs[c, :, ATAB0:ATAB0 + 2 * RQ] = _ATABS[q][0:128]
        blobs[c, :, ATAB1:ATAB1 + 2 * RQ] = _ATABS[q][128:256]
        blobs[c, :, BTAB0:BTAB0 + 3 * S] = _BTAB[0:128]
        blobs[c, :, BTAB1:BTAB1 + 3 * S] = _BTAB[128:256]
    return blobs


def _host_interp(F, trajectory):
    """F: [B, G1H, S] complex64 (g1 = 0..S/2; col h -> freq _G2F[h])."""
    kx = trajectory[0].astype(np.float64)
    ky = trajectory[1].astype(np.float64)
    eta1 = kx * S / (2 * np.pi)
    eta2 = ky * S / (2 * np.pi)
    a0 = np.floor(eta1).astype(int) - J // 2 + 1
    b0 = np.floor(eta2).astype(int) - J // 2 + 1
    js = np.arange(J)
    w1 = _es_kernel(eta1[:, None] - (a0[:, None] + js[None, :]))  # [M, J]
    w2 = _es_kernel(eta2[:, None] - (b0[:, None] + js[None, :]))
    gg1 = (a0[:, None] + js[None, :]) % S                         # [M, J]
    gg2 = (b0[:, None] + js[None, :]) % S

    # full F grid [B, S, S] indexed by (g1 mod S, g2 mod S)
    Ffull = np.zeros((B, S, S), dtype=np.complex64)
    q2 = (_G2F % S)
    rows = np.arange(G1H)
    Ffull[:, rows[:, None] % S, q2[None, :]] = F
    neg = np.arange(1, S // 2)
    mirr = (S - np.arange(S)) % S
    Ffull[:, (-neg) % S, :] = np.conj(Ffull[:, neg][:, :, mirr])

    vals = Ffull[:, gg1[:, :, None], gg2[:, None, :]]             # [B, M, J, J]
    w = (w1[:, :, None] * w2[:, None, :]).astype(np.float32)      # [M, J, J]
    return (vals * w[None]).sum(axis=(2, 3)).astype(np.complex64)


def kernel(image, trajectory):
    from concourse.bass_utils import run_bass_kernel_spmd

    if 'nc' not in _CACHE:
        _CACHE['nc'] = _build()
    nc = _CACHE['nc']

    image = np.ascontiguousarray(np.asarray(image, dtype=np.float32))
    trajectory = np.ascontiguousarray(np.asarray(trajectory, dtype=np.float32))
    blobs = _host_prep(image, trajectory)

    in_maps = [{"blob": np.ascontiguousarray(blobs[c])} for c in range(NCORES)]
    res = run_bass_kernel_spmd(nc, in_maps, core_ids=list(range(NCORES)))

    F = np.zeros((B, G1H, S), dtype=np.complex64)
    for c in range(NCORES):
        b, q = c // NQ, c % NQ
        o = res.results[c]["out"]                      # [128, 2*S] f32
        lo = q * RQ
        hi = min(lo + RQ, G1H)
        F[b, lo:hi, :] = o[0:hi - lo, 0:S] + 1j * o[0:hi - lo, S:2 * S]

    return _host_interp(F, trajectory)


# revision 15
# speedup vs baseline: 2.4925x; 1.0065x over previous
"""Type-2 NUFFT (image -> non-uniform k-space) on 8 Trainium2 NeuronCores.

kspace[b,m] = sum_{x,y} image[b,x,y] * exp(-i*(kx_m*(x-128) + ky_m*(y-128)))

Gridding (NUFFT) formulation: with an exponential-of-semicircle kernel psi
(width J, oversampled grid S > N),

  exp(-i*k*xt) ~= (1/D(xt)) * sum_g psi(k*S/2pi - g) * exp(-i*2pi*g*xt/S)

so   kspace[b,m] ~= sum_{JxJ window} F[b,g1,g2] * w1[m]*w2[m]
with F = dense DFT of the deapodized image on the S x S grid.

Work split:
  device: the dense DFT (all the heavy FLOPs), as two matmul passes
     A[g1,y] = sum_x imgd[x,y] e1[g1,x]     (stage A, complex via 2 blocks)
     F[g1,g2] = sum_y A[g1,y] e2[g2,y]      (stage B)
   sharded over 8 cores: core = (batch, g1-half of the Hermitian-half
   range [0, S/2], g2-half).  Each core outputs F f32 [97, 2*(S/2)].
  host: deapodization, trig tables, and the O(M*J^2) window interpolation
   (including Hermitian reconstruction of negative g1 rows).
"""

import sys

if '/opt/trn_rl_repo' not in sys.path:
    sys.path.insert(0, '/opt/trn_rl_repo')

import numpy as np
import ml_dtypes

B, NX, NY, M, NCORES = 2, 256, 256, 16384, 8

S = 384                  # oversampled grid (sigma = 1.5)
J = 6                    # interp kernel width (host side only)
G1H = S // 2 + 1         # Hermitian half rows: 193
RQ = 97                  # g1 rows per core (2*97 = 194 >= 193; tail padded)
SH = S // 2              # g2 cols per core: 192
BETA = np.pi * (J / 2.0) * (2.0 - 256.0 / S)

# blob layout (bf16 cols):
#   [img_x0(256) | atab_x0(2*RQ) | img_x1(256) | atab_x1(2*RQ) |
#    btab_y0(2*SH) | btab_y1(2*SH)]        btab = [Cy | Sy] (g2-half)
IMG0, ATAB0 = 0, 256
IMG1 = 256 + 2 * RQ
ATAB1 = IMG1 + 256
BTAB0 = ATAB1 + 2 * RQ
BTAB1 = BTAB0 + 2 * SH
BLOB_COLS = BTAB1 + 2 * SH

_CACHE = {}


def _es_kernel(z):
    c = J / 2.0
    out = np.zeros_like(z)
    m = np.abs(z) < c
    out[m] = np.exp(BETA * (np.sqrt(1.0 - (z[m] / c) ** 2) - 1.0))
    return out


def _deapod():
    """D(xt) = continuous FT of psi at xt/S cycles (trapezoid quadrature)."""
    c = J / 2.0
    xt = (np.arange(NX) - NX // 2).astype(np.float64)
    zq = np.linspace(-c, c, 4001)
    pz = _es_kernel(zq)
    D = np.trapezoid(pz[None, :] * np.exp(1j * 2 * np.pi * zq[None, :]
                                          * xt[:, None] / S), zq, axis=1).real
    return D


_DEAPOD = _deapod()                               # [256]
_XT = (np.arange(NX) - NX // 2).astype(np.float64)
_G2F = ((np.arange(S) + S // 2) % S - S // 2)     # col h -> g2 freq


def _tables():
    """Static device trig tables (bf16): per-half A tables + per-half B."""
    bf = ml_dtypes.bfloat16
    atabs = []
    for q in range(2):
        g = np.minimum(np.arange(q * RQ, (q + 1) * RQ), G1H - 1)
        ph = 2 * np.pi * g[None, :] * _XT[:, None] / S        # [256, RQ]
        atabs.append(np.concatenate([np.cos(ph), -np.sin(ph)],
                                    axis=1).astype(bf))       # [256, 2*RQ]
    btabs = []
    for q in range(2):
        g2 = _G2F[q * SH:(q + 1) * SH]
        ph2 = 2 * np.pi * g2[None, :] * _XT[:, None] / S      # [256, SH]
        btabs.append(np.concatenate([np.cos(ph2), np.sin(ph2)],
                                    axis=1).astype(bf))       # [256, 2*SH]
    return atabs, btabs


_ATABS, _BTABS = _tables()


def _build(warm=0):
    import concourse.bacc as bacc
    import concourse.mybir as mybir
    from concourse.tile import TileContext

    f32 = mybir.dt.float32
    bf16 = mybir.dt.bfloat16
    A = mybir.AluOpType

    nc = bacc.Bacc("TRN2", target_bir_lowering=False, debug=False)

    blob = nc.dram_tensor("blob", [128, BLOB_COLS], bf16, kind="ExternalInput")
    out = nc.dram_tensor("out", [128, 2 * SH], f32, kind="ExternalOutput")

    TW = 2 * RQ      # A cols: [Ar | Ai]
    with TileContext(nc) as tc:
        with tc.tile_pool(name="const", bufs=1) as cpool, \
             tc.tile_pool(name="work", bufs=1) as wpool, \
             tc.tile_pool(name="ps", bufs=1, space="PSUM") as ps:

            bsb = cpool.tile([128, BLOB_COLS], bf16, name="blob")

            # DMA chunks in consumption order
            bounds = [0, BTAB0, BTAB1, BLOB_COLS]
            for i in range(len(bounds) - 1):
                cs = slice(bounds[i], bounds[i + 1])
                nc.sync.dma_start(bsb[:, cs], blob[:, cs])

            if warm:
                wsrc = cpool.tile([128, 512], bf16, name="warmsrc")
                nc.gpsimd.memset(wsrc[:, :], 1.0)
                wab = ps.tile([128, 512], f32, tag="warm", bufs=1)
                for _ in range(warm):
                    nc.tensor.matmul(wab[:, :], wsrc[:, 0:128], wsrc[:, :],
                                     start=True, stop=True)

            # stage A: A^T[y, (Ar|Ai)] per y-chunk, contract x (2 chunks);
            # evict adds a negated-Ar block: asb = [Ar | Ai | -Ar]
            asb = []
            for yc in range(2):
                aps = ps.tile([128, TW], f32, tag="aps", bufs=2)
                for xc in range(2):
                    imgc = IMG0 if xc == 0 else IMG1
                    atc = ATAB0 if xc == 0 else ATAB1
                    nc.tensor.matmul(aps[:, :],
                                     bsb[:, imgc + yc * 128:imgc + yc * 128 + 128],
                                     bsb[:, atc:atc + TW],
                                     start=(xc == 0), stop=(xc == 1))
                a = wpool.tile([128, TW + RQ], bf16, tag=f"asb{yc}")
                eng = nc.vector if yc == 0 else nc.scalar
                if yc == 0:
                    eng.tensor_scalar(a[:, 0:TW], aps[:, :], scalar1=1.0,
                                      scalar2=0.0, op0=A.mult, op1=A.add)
                    eng.tensor_scalar(a[:, TW:TW + RQ], aps[:, 0:RQ],
                                      scalar1=-1.0, scalar2=0.0,
                                      op0=A.mult, op1=A.add)
                else:
                    nc.scalar.activation(a[:, 0:TW], aps[:, :],
                                         mybir.ActivationFunctionType.Copy)
                    nc.scalar.activation(a[:, TW:TW + RQ], aps[:, 0:RQ],
                                         mybir.ActivationFunctionType.Copy,
                                         scale=-1.0)
                asb.append(a)

            # stage B: Fr/Fi (separate PSUM banks); btab_yc = [Cy | Sy]
            fr = ps.tile([128, SH], f32, tag="fr", bufs=1)
            fi = ps.tile([128, SH], f32, tag="fi", bufs=1)
            for yc in range(2):
                bt = BTAB0 if yc == 0 else BTAB1
                ar = asb[yc][:, 0:RQ]
                ai = asb[yc][:, RQ:TW]
                mar = asb[yc][:, TW:TW + RQ]
                st, sp = (yc == 0), (yc == 1)
                # Fr = Ar*Cy + Ai*Sy ; Fi = Ai*Cy + (-Ar)*Sy
                nc.tensor.matmul(fr[0:RQ, :], ar, bsb[:, bt:bt + SH],
                                 start=st, stop=False)
                nc.tensor.matmul(fr[0:RQ, :], ai, bsb[:, bt + SH:bt + 2 * SH],
                                 start=False, stop=sp)
                nc.tensor.matmul(fi[0:RQ, :], ai, bsb[:, bt:bt + SH],
                                 start=st, stop=False)
                nc.tensor.matmul(fi[0:RQ, :], mar, bsb[:, bt + SH:bt + 2 * SH],
                                 start=False, stop=sp)

            # evict F (f32) on DVE/Act in parallel; each engine then issues
            # its own out-DMA so nothing serializes on SP
            fsb = cpool.tile([128, 2 * SH], f32, name="fsb")
            nc.vector.tensor_scalar(fsb[0:RQ, 0:SH], fr[0:RQ, :], scalar1=1.0,
                                    scalar2=0.0, op0=A.mult, op1=A.add)
            nc.sync.dma_start(out[0:RQ, 0:SH], fsb[0:RQ, 0:SH])
            nc.scalar.copy(fsb[0:RQ, SH:2 * SH], fi[0:RQ, :])
            nc.scalar.dma_start(out[0:RQ, SH:2 * SH], fsb[0:RQ, SH:2 * SH])

    nc.compile()
    return nc


def _host_prep(image, trajectory):
    bf = ml_dtypes.bfloat16
    imgd = (image / (_DEAPOD[None, :, None] * _DEAPOD[None, None, :])
            ).astype(bf)                                   # [B, 256, 256]
    blobs = np.zeros((NCORES, 128, BLOB_COLS), dtype=bf)
    for c in range(NCORES):
        b, q1, q2 = c // 4, (c // 2) % 2, c % 2
        blobs[c, :, IMG0:IMG0 + 256] = imgd[b, 0:128, :]
        blobs[c, :, IMG1:IMG1 + 256] = imgd[b, 128:256, :]
        blobs[c, :, ATAB0:ATAB0 + 2 * RQ] = _ATABS[q1][0:128]
        blobs[c, :, ATAB1:ATAB1 + 2 * RQ] = _ATABS[q1][128:256]
        blobs[c, :, BTAB0:BTAB0 + 2 * SH] = _BTABS[q2][0:128]
        blobs[c, :, BTAB1:BTAB1 + 2 * SH] = _BTABS[q2][128:256]
    return blobs


def _host_interp(F, trajectory):
    """F: [B, G1H, S] complex64 (g1 = 0..S/2; col h -> freq _G2F[h])."""
    kx = trajectory[0].astype(np.float64)
    ky = trajectory[1].astype(np.float64)
    eta1 = kx * S / (2 * np.pi)
    eta2 = ky * S / (2 * np.pi)
    a0 = np.floor(eta1).astype(int) - J // 2 + 1
    b0 = np.floor(eta2).astype(int) - J // 2 + 1
    js = np.arange(J)
    w1 = _es_kernel(eta1[:, None] - (a0[:, None] + js[None, :]))  # [M, J]
    w2 = _es_kernel(eta2[:, None] - (b0[:, None] + js[None, :]))
    gg1 = (a0[:, None] + js[None, :]) % S                         # [M, J]
    gg2 = (b0[:, None] + js[None, :]) % S

    # full F grid [B, S, S] indexed by (g1 mod S, g2 mod S)
    Ffull = np.zeros((B, S, S), dtype=np.complex64)
    q2 = (_G2F % S)
    rows = np.arange(G1H)
    Ffull[:, rows[:, None] % S, q2[None, :]] = F
    neg = np.arange(1, S // 2)
    mirr = (S - np.arange(S)) % S
    Ffull[:, (-neg) % S, :] = np.conj(Ffull[:, neg][:, :, mirr])

    vals = Ffull[:, gg1[:, :, None], gg2[:, None, :]]             # [B, M, J, J]
    w = (w1[:, :, None] * w2[:, None, :]).astype(np.float32)      # [M, J, J]
    return (vals * w[None]).sum(axis=(2, 3)).astype(np.complex64)


def kernel(image, trajectory):
    from concourse.bass_utils import run_bass_kernel_spmd

    if 'nc' not in _CACHE:
        _CACHE['nc'] = _build()
    nc = _CACHE['nc']

    image = np.ascontiguousarray(np.asarray(image, dtype=np.float32))
    trajectory = np.ascontiguousarray(np.asarray(trajectory, dtype=np.float32))
    blobs = _host_prep(image, trajectory)

    in_maps = [{"blob": np.ascontiguousarray(blobs[c])} for c in range(NCORES)]
    res = run_bass_kernel_spmd(nc, in_maps, core_ids=list(range(NCORES)))

    F = np.zeros((B, G1H, S), dtype=np.complex64)
    for c in range(NCORES):
        b, q1, q2 = c // 4, (c // 2) % 2, c % 2
        o = res.results[c]["out"]                      # [128, 2*SH] f32
        lo = q1 * RQ
        hi = min(lo + RQ, G1H)
        F[b, lo:hi, q2 * SH:(q2 + 1) * SH] = (o[0:hi - lo, 0:SH]
                                              + 1j * o[0:hi - lo, SH:2 * SH])

    return _host_interp(F, trajectory)


# revision 23
# speedup vs baseline: 2.7687x; 1.1108x over previous
"""Type-2 NUFFT (image -> non-uniform k-space) on 8 Trainium2 NeuronCores.

kspace[b,m] = sum_{x,y} image[b,x,y] * exp(-i*(kx_m*(x-128) + ky_m*(y-128)))

Gridding (NUFFT) formulation: with an exponential-of-semicircle kernel psi
(width J, oversampled grid S > N),

  exp(-i*k*xt) ~= (1/D(xt)) * sum_g psi(k*S/2pi - g) * exp(-i*2pi*g*xt/S)

so   kspace[b,m] ~= sum_{JxJ window} F[b,g1,g2] * w1[m]*w2[m]
with F = dense DFT of the deapodized image on the S x S grid.

Work split:
  device: the dense DFT (all the heavy FLOPs), as two matmul passes
     A[g1,y] = sum_x imgd[x,y] e1[g1,x]     (stage A, complex via 2 blocks)
     F[g1,g2] = sum_y A[g1,y] e2[g2,y]      (stage B)
   sharded over 8 cores: core = (batch, g1-half of the Hermitian-half
   range [0, S/2], g2-half).  Each core outputs F f32 [97, 2*(S/2)].
  host: deapodization, trig tables, and the O(M*J^2) window interpolation
   (including Hermitian reconstruction of negative g1 rows).
"""

import sys

if '/opt/trn_rl_repo' not in sys.path:
    sys.path.insert(0, '/opt/trn_rl_repo')

import numpy as np
import ml_dtypes

B, NX, NY, M, NCORES = 2, 256, 256, 16384, 8

S = 384                  # oversampled grid (sigma = 1.5)
J = 6                    # interp kernel width (host side only)
G1H = S // 2 + 1         # Hermitian half rows: 193
RQ = 97                  # g1 rows per core (2*97 = 194 >= 193; tail padded)
SH = S // 2              # g2 cols per core: 192
BETA = np.pi * (J / 2.0) * (2.0 - 256.0 / S)

# blob layout (bf16 cols):
#   [img_x0(256) | atab_x0(2*RQ) | img_x1(256) | atab_x1(2*RQ) |
#    btab_y0(2*SH) | btab_y1(2*SH)]        btab = [Cy | Sy] (g2-half)
IMG0, ATAB0 = 0, 256
IMG1 = 256 + 2 * RQ
ATAB1 = IMG1 + 256
BTAB0 = ATAB1 + 2 * RQ
BTAB1 = BTAB0 + 2 * SH
BLOB_COLS = BTAB1 + 2 * SH

_CACHE = {}


def _es_kernel(z):
    c = J / 2.0
    out = np.zeros_like(z)
    m = np.abs(z) < c
    out[m] = np.exp(BETA * (np.sqrt(1.0 - (z[m] / c) ** 2) - 1.0))
    return out


def _deapod():
    """D(xt) = continuous FT of psi at xt/S cycles (trapezoid quadrature)."""
    c = J / 2.0
    xt = (np.arange(NX) - NX // 2).astype(np.float64)
    zq = np.linspace(-c, c, 4001)
    pz = _es_kernel(zq)
    D = np.trapezoid(pz[None, :] * np.exp(1j * 2 * np.pi * zq[None, :]
                                          * xt[:, None] / S), zq, axis=1).real
    return D


_DEAPOD = _deapod()                               # [256]
_XT = (np.arange(NX) - NX // 2).astype(np.float64)
_G2F = ((np.arange(S) + S // 2) % S - S // 2)     # col h -> g2 freq


def _tables():
    """Static device trig tables (bf16): per-half A tables + per-half B."""
    bf = ml_dtypes.bfloat16
    atabs = []
    for q in range(2):
        g = np.minimum(np.arange(q * RQ, (q + 1) * RQ), G1H - 1)
        ph = 2 * np.pi * g[None, :] * _XT[:, None] / S        # [256, RQ]
        atabs.append(np.concatenate([np.cos(ph), -np.sin(ph)],
                                    axis=1).astype(bf))       # [256, 2*RQ]
    btabs = []
    for q in range(2):
        g2 = _G2F[q * SH:(q + 1) * SH]
        ph2 = 2 * np.pi * g2[None, :] * _XT[:, None] / S      # [256, SH]
        btabs.append(np.concatenate([np.cos(ph2), np.sin(ph2)],
                                    axis=1).astype(bf))       # [256, 2*SH]
    return atabs, btabs


_ATABS, _BTABS = _tables()


def _build(warm=4, fi_out='act'):
    import concourse.bacc as bacc
    import concourse.mybir as mybir
    from concourse.tile import TileContext

    f32 = mybir.dt.float32
    bf16 = mybir.dt.bfloat16
    A = mybir.AluOpType

    nc = bacc.Bacc("TRN2", target_bir_lowering=False, debug=False)

    blob = nc.dram_tensor("blob", [128, BLOB_COLS], bf16, kind="ExternalInput")
    out = nc.dram_tensor("out", [128, 2 * SH], bf16, kind="ExternalOutput")

    TW = 2 * RQ      # A cols: [Ar | Ai]
    with TileContext(nc) as tc:
        with tc.tile_pool(name="const", bufs=1) as cpool, \
             tc.tile_pool(name="work", bufs=1) as wpool, \
             tc.tile_pool(name="ps", bufs=1, space="PSUM") as ps:

            bsb = cpool.tile([128, BLOB_COLS], bf16, name="blob")

            # DMA chunks in consumption order (A inputs, then both btabs --
            # fewer copies beat finer pipelining: each copy re-pays the
            # HWDGE slot + descriptor-gen latency)
            bounds = [0, BTAB0, BLOB_COLS]
            for i in range(len(bounds) - 1):
                cs = slice(bounds[i], bounds[i + 1])
                nc.sync.dma_start(bsb[:, cs], blob[:, cs])

            if warm:
                wsrc = cpool.tile([128, 512], bf16, name="warmsrc")
                nc.gpsimd.memset(wsrc[:, :], 1.0)
                wab = ps.tile([128, 512], f32, tag="warm", bufs=1)
                for _ in range(warm):
                    nc.tensor.matmul(wab[:, :], wsrc[:, 0:128], wsrc[:, :],
                                     start=True, stop=True)

            # stage A: A^T[y, (Ar|Ai)] per y-chunk, contract x (2 chunks);
            # evict adds a negated-Ar block: asb = [Ar | Ai | -Ar]
            asb = []
            for yc in range(2):
                aps = ps.tile([128, TW], f32, tag="aps", bufs=2)
                for xc in range(2):
                    imgc = IMG0 if xc == 0 else IMG1
                    atc = ATAB0 if xc == 0 else ATAB1
                    nc.tensor.matmul(aps[:, :],
                                     bsb[:, imgc + yc * 128:imgc + yc * 128 + 128],
                                     bsb[:, atc:atc + TW],
                                     start=(xc == 0), stop=(xc == 1))
                a = wpool.tile([128, TW + RQ], bf16, tag=f"asb{yc}")
                eng = nc.vector if yc == 0 else nc.scalar
                if yc == 0:
                    eng.tensor_scalar(a[:, 0:TW], aps[:, :], scalar1=1.0,
                                      scalar2=0.0, op0=A.mult, op1=A.add)
                    eng.tensor_scalar(a[:, TW:TW + RQ], aps[:, 0:RQ],
                                      scalar1=-1.0, scalar2=0.0,
                                      op0=A.mult, op1=A.add)
                else:
                    nc.scalar.activation(a[:, 0:TW], aps[:, :],
                                         mybir.ActivationFunctionType.Copy)
                    nc.scalar.activation(a[:, TW:TW + RQ], aps[:, 0:RQ],
                                         mybir.ActivationFunctionType.Copy,
                                         scale=-1.0)
                asb.append(a)

            # stage B: Fr/Fi (separate PSUM banks); btab_yc = [Cy | Sy]
            fr = ps.tile([128, SH], f32, tag="fr", bufs=1)
            fi = ps.tile([128, SH], f32, tag="fi", bufs=1)
            for yc in range(2):
                bt = BTAB0 if yc == 0 else BTAB1
                ar = asb[yc][:, 0:RQ]
                ai = asb[yc][:, RQ:TW]
                mar = asb[yc][:, TW:TW + RQ]
                st, sp = (yc == 0), (yc == 1)
                # Fr = Ar*Cy + Ai*Sy ; Fi = Ai*Cy + (-Ar)*Sy
                nc.tensor.matmul(fr[0:RQ, :], ar, bsb[:, bt:bt + SH],
                                 start=st, stop=False)
                nc.tensor.matmul(fr[0:RQ, :], ai, bsb[:, bt + SH:bt + 2 * SH],
                                 start=False, stop=sp)
                nc.tensor.matmul(fi[0:RQ, :], ai, bsb[:, bt:bt + SH],
                                 start=st, stop=False)
                nc.tensor.matmul(fi[0:RQ, :], mar, bsb[:, bt + SH:bt + 2 * SH],
                                 start=False, stop=sp)

            # evict F (f32) on DVE/Act in parallel, then one combined
            # out-DMA from SP (two copies would serialize on HWDGE)
            fsb = cpool.tile([128, 2 * SH], bf16, name="fsb")
            nc.vector.tensor_scalar(fsb[0:RQ, 0:SH], fr[0:RQ, :], scalar1=1.0,
                                    scalar2=0.0, op0=A.mult, op1=A.add)
            nc.scalar.copy(fsb[0:RQ, SH:2 * SH], fi[0:RQ, :])
            nc.sync.dma_start(out[0:RQ, :], fsb[0:RQ, :])

    nc.compile()
    return nc


def _host_prep(image, trajectory):
    bf = ml_dtypes.bfloat16
    imgd = (image / (_DEAPOD[None, :, None] * _DEAPOD[None, None, :])
            ).astype(bf)                                   # [B, 256, 256]
    blobs = np.zeros((NCORES, 128, BLOB_COLS), dtype=bf)
    for c in range(NCORES):
        b, q1, q2 = c // 4, (c // 2) % 2, c % 2
        blobs[c, :, IMG0:IMG0 + 256] = imgd[b, 0:128, :]
        blobs[c, :, IMG1:IMG1 + 256] = imgd[b, 128:256, :]
        blobs[c, :, ATAB0:ATAB0 + 2 * RQ] = _ATABS[q1][0:128]
        blobs[c, :, ATAB1:ATAB1 + 2 * RQ] = _ATABS[q1][128:256]
        blobs[c, :, BTAB0:BTAB0 + 2 * SH] = _BTABS[q2][0:128]
        blobs[c, :, BTAB1:BTAB1 + 2 * SH] = _BTABS[q2][128:256]
    return blobs


def _host_interp(F, trajectory):
    """F: [B, G1H, S] complex64 (g1 = 0..S/2; col h -> freq _G2F[h])."""
    kx = trajectory[0].astype(np.float64)
    ky = trajectory[1].astype(np.float64)
    eta1 = kx * S / (2 * np.pi)
    eta2 = ky * S / (2 * np.pi)
    a0 = np.floor(eta1).astype(int) - J // 2 + 1
    b0 = np.floor(eta2).astype(int) - J // 2 + 1
    js = np.arange(J)
    w1 = _es_kernel(eta1[:, None] - (a0[:, None] + js[None, :]))  # [M, J]
    w2 = _es_kernel(eta2[:, None] - (b0[:, None] + js[None, :]))
    gg1 = (a0[:, None] + js[None, :]) % S                         # [M, J]
    gg2 = (b0[:, None] + js[None, :]) % S

    # full F grid [B, S, S] indexed by (g1 mod S, g2 mod S)
    Ffull = np.zeros((B, S, S), dtype=np.complex64)
    q2 = (_G2F % S)
    rows = np.arange(G1H)
    Ffull[:, rows[:, None] % S, q2[None, :]] = F
    neg = np.arange(1, S // 2)
    mirr = (S - np.arange(S)) % S
    Ffull[:, (-neg) % S, :] = np.conj(Ffull[:, neg][:, :, mirr])

    vals = Ffull[:, gg1[:, :, None], gg2[:, None, :]]             # [B, M, J, J]
    w = (w1[:, :, None] * w2[:, None, :]).astype(np.float32)      # [M, J, J]
    return (vals * w[None]).sum(axis=(2, 3)).astype(np.complex64)


def kernel(image, trajectory):
    from concourse.bass_utils import run_bass_kernel_spmd

    if 'nc' not in _CACHE:
        _CACHE['nc'] = _build()
    nc = _CACHE['nc']

    image = np.ascontiguousarray(np.asarray(image, dtype=np.float32))
    trajectory = np.ascontiguousarray(np.asarray(trajectory, dtype=np.float32))
    blobs = _host_prep(image, trajectory)

    in_maps = [{"blob": np.ascontiguousarray(blobs[c])} for c in range(NCORES)]
    res = run_bass_kernel_spmd(nc, in_maps, core_ids=list(range(NCORES)))

    F = np.zeros((B, G1H, S), dtype=np.complex64)
    for c in range(NCORES):
        b, q1, q2 = c // 4, (c // 2) % 2, c % 2
        o = res.results[c]["out"].astype(np.float32)   # [128, 2*SH]
        lo = q1 * RQ
        hi = min(lo + RQ, G1H)
        F[b, lo:hi, q2 * SH:(q2 + 1) * SH] = (o[0:hi - lo, 0:SH]
                                              + 1j * o[0:hi - lo, SH:2 * SH])

    return _host_interp(F, trajectory)


# revision 29
# speedup vs baseline: 2.8469x; 1.0282x over previous
"""Type-2 NUFFT (image -> non-uniform k-space) on 8 Trainium2 NeuronCores.

kspace[b,m] = sum_{x,y} image[b,x,y] * exp(-i*(kx_m*(x-128) + ky_m*(y-128)))

Gridding (NUFFT) formulation: with an exponential-of-semicircle kernel psi
(width J, oversampled grid S > N),

  exp(-i*k*xt) ~= (1/D(xt)) * sum_g psi(k*S/2pi - g) * exp(-i*2pi*g*xt/S)

so   kspace[b,m] ~= sum_{JxJ window} F[b,g1,g2] * w1[m]*w2[m]
with F = dense DFT of the deapodized image on the S x S grid.

Work split:
  device: the dense DFT (all the heavy FLOPs), as two matmul passes
     A[g1,y] = sum_x imgd[x,y] e1[g1,x]     (stage A, complex via 2 blocks)
     F[g1,g2] = sum_y A[g1,y] e2[g2,y]      (stage B)
   sharded over 8 cores: core = (batch, g1-half of the Hermitian-half
   range [0, S/2], g2-half).  Each core outputs F f32 [97, 2*(S/2)].
  host: deapodization, trig tables, and the O(M*J^2) window interpolation
   (including Hermitian reconstruction of negative g1 rows).
"""

import sys

if '/opt/trn_rl_repo' not in sys.path:
    sys.path.insert(0, '/opt/trn_rl_repo')

import numpy as np
import ml_dtypes

B, NX, NY, M, NCORES = 2, 256, 256, 16384, 8

S = 384                  # oversampled grid (sigma = 1.5)
J = 6                    # interp kernel width (host side only)
G1H = S // 2 + 1         # Hermitian half rows: 193
RQ = 97                  # g1 rows per core (2*97 = 194 >= 193; tail padded)
SH = S // 2              # g2 cols per core: 192
BETA = np.pi * (J / 2.0) * (2.0 - 256.0 / S)

# blob layout (bf16 cols):
#   [img_x0(256) | atab_x0(2*RQ) | img_x1(256) | atab_x1(2*RQ) |
#    btab_y0(2*SH) | btab_y1(2*SH)]        btab = [Cy | Sy] (g2-half)
IMG0, ATAB0 = 0, 256
IMG1 = 256 + 2 * RQ
ATAB1 = IMG1 + 256
BTAB0 = ATAB1 + 2 * RQ
BTAB1 = BTAB0 + 3 * SH
BLOB_COLS = BTAB1 + 3 * SH

_CACHE = {}


def _es_kernel(z):
    c = J / 2.0
    out = np.zeros_like(z)
    m = np.abs(z) < c
    out[m] = np.exp(BETA * (np.sqrt(1.0 - (z[m] / c) ** 2) - 1.0))
    return out


def _deapod():
    """D(xt) = continuous FT of psi at xt/S cycles (trapezoid quadrature)."""
    c = J / 2.0
    xt = (np.arange(NX) - NX // 2).astype(np.float64)
    zq = np.linspace(-c, c, 4001)
    pz = _es_kernel(zq)
    D = np.trapezoid(pz[None, :] * np.exp(1j * 2 * np.pi * zq[None, :]
                                          * xt[:, None] / S), zq, axis=1).real
    return D


_DEAPOD = _deapod()                               # [256]
_XT = (np.arange(NX) - NX // 2).astype(np.float64)
_G2F = ((np.arange(S) + S // 2) % S - S // 2)     # col h -> g2 freq


def _tables():
    """Static device trig tables (bf16): per-half A tables + per-half B."""
    bf = ml_dtypes.bfloat16
    atabs = []
    for q in range(2):
        g = np.minimum(np.arange(q * RQ, (q + 1) * RQ), G1H - 1)
        ph = 2 * np.pi * g[None, :] * _XT[:, None] / S        # [256, RQ]
        atabs.append(np.concatenate([np.cos(ph), -np.sin(ph)],
                                    axis=1).astype(bf))       # [256, 2*RQ]
    btabs = []
    for q in range(2):
        g2 = _G2F[q * SH:(q + 1) * SH]
        ph2 = 2 * np.pi * g2[None, :] * _XT[:, None] / S      # [256, SH]
        sy = np.sin(ph2)
        btabs.append(np.concatenate([np.cos(ph2), sy, -sy],
                                    axis=1).astype(bf))       # [256, 3*SH]
    return atabs, btabs


_ATABS, _BTABS = _tables()


def _build(warm=4, fi_out='act'):
    import concourse.bacc as bacc
    import concourse.mybir as mybir
    from concourse.tile import TileContext

    f32 = mybir.dt.float32
    bf16 = mybir.dt.bfloat16
    A = mybir.AluOpType

    nc = bacc.Bacc("TRN2", target_bir_lowering=False, debug=False)

    blob = nc.dram_tensor("blob", [128, BLOB_COLS], bf16, kind="ExternalInput")
    out = nc.dram_tensor("out", [128, 2 * SH], bf16, kind="ExternalOutput")

    TW = 2 * RQ      # A cols: [Ar | Ai]
    with TileContext(nc) as tc:
        with tc.tile_pool(name="const", bufs=1) as cpool, \
             tc.tile_pool(name="ps", bufs=1, space="PSUM") as ps:
            wpool = cpool

            bsb = cpool.tile([128, BLOB_COLS], bf16, name="blob")
            fsb0 = cpool.tile([128, 2 * SH], bf16, name="fsb")

            # DMA chunks in consumption order (A inputs, then both btabs --
            # fewer copies beat finer pipelining: each copy re-pays the
            # HWDGE slot + descriptor-gen latency)
            bounds = [0, BTAB0, BLOB_COLS]
            for i in range(len(bounds) - 1):
                cs = slice(bounds[i], bounds[i + 1])
                nc.sync.dma_start(bsb[:, cs], blob[:, cs])

            if warm:
                # p-state warm-up: dummy matmuls reading fsb before it is
                # written (values irrelevant, wab is never read)
                wab = ps.tile([128, 384], f32, tag="warm", bufs=1)
                for _ in range(warm):
                    nc.tensor.matmul(wab[:, :], fsb0[:, 0:128],
                                     fsb0[:, 0:384], start=True,
                                     stop=True)

            # stage A: A^T[y, (Ar|Ai)] per y-chunk, contract x (2 chunks);
            # evict adds a negated-Ar block: asb = [Ar | Ai | -Ar]
            asb = []
            for yc in range(2):
                aps = ps.tile([128, TW], f32, tag="aps", bufs=2)
                for xc in range(2):
                    imgc = IMG0 if xc == 0 else IMG1
                    atc = ATAB0 if xc == 0 else ATAB1
                    nc.tensor.matmul(aps[:, :],
                                     bsb[:, imgc + yc * 128:imgc + yc * 128 + 128],
                                     bsb[:, atc:atc + TW],
                                     start=(xc == 0), stop=(xc == 1))
                a = wpool.tile([128, TW], bf16, tag=f"asb{yc}")
                if yc == 0:
                    nc.vector.tensor_scalar(a[:, :], aps[:, :], scalar1=1.0,
                                            scalar2=0.0, op0=A.mult,
                                            op1=A.add)
                else:
                    nc.scalar.copy(a[:, :], aps[:, :])
                asb.append(a)

            # stage B: Fr/Fi (separate PSUM banks); btab_yc = [Cy | Sy]
            fr = ps.tile([128, SH], f32, tag="fr", bufs=1)
            fi = ps.tile([128, SH], f32, tag="fi", bufs=1)
            for yc in range(2):
                bt = BTAB0 if yc == 0 else BTAB1
                ar = asb[yc][:, 0:RQ]
                ai = asb[yc][:, RQ:TW]
                st, sp = (yc == 0), (yc == 1)
                # Fr = Ar*Cy + Ai*Sy ; Fi = Ai*Cy + Ar*(-Sy)
                nc.tensor.matmul(fr[0:RQ, :], ar, bsb[:, bt:bt + SH],
                                 start=st, stop=False)
                nc.tensor.matmul(fr[0:RQ, :], ai, bsb[:, bt + SH:bt + 2 * SH],
                                 start=False, stop=sp)
                nc.tensor.matmul(fi[0:RQ, :], ai, bsb[:, bt:bt + SH],
                                 start=st, stop=False)
                nc.tensor.matmul(fi[0:RQ, :], ar, bsb[:, bt + 2 * SH:bt + 3 * SH],
                                 start=False, stop=sp)

            # evict F (f32) on DVE/Act in parallel, then one combined
            # out-DMA from SP (two copies would serialize on HWDGE)
            fsb = fsb0
            nc.vector.tensor_scalar(fsb[0:RQ, 0:SH], fr[0:RQ, :], scalar1=1.0,
                                    scalar2=0.0, op0=A.mult, op1=A.add)
            nc.scalar.copy(fsb[0:RQ, SH:2 * SH], fi[0:RQ, :])
            nc.sync.dma_start(out[0:RQ, :], fsb[0:RQ, :])

    nc.compile()
    return nc


def _host_prep(image, trajectory):
    bf = ml_dtypes.bfloat16
    imgd = (image / (_DEAPOD[None, :, None] * _DEAPOD[None, None, :])
            ).astype(bf)                                   # [B, 256, 256]
    blobs = np.zeros((NCORES, 128, BLOB_COLS), dtype=bf)
    for c in range(NCORES):
        b, q1, q2 = c // 4, (c // 2) % 2, c % 2
        blobs[c, :, IMG0:IMG0 + 256] = imgd[b, 0:128, :]
        blobs[c, :, IMG1:IMG1 + 256] = imgd[b, 128:256, :]
        blobs[c, :, ATAB0:ATAB0 + 2 * RQ] = _ATABS[q1][0:128]
        blobs[c, :, ATAB1:ATAB1 + 2 * RQ] = _ATABS[q1][128:256]
        blobs[c, :, BTAB0:BTAB0 + 3 * SH] = _BTABS[q2][0:128]
        blobs[c, :, BTAB1:BTAB1 + 3 * SH] = _BTABS[q2][128:256]
    return blobs


def _host_interp(F, trajectory):
    """F: [B, G1H, S] complex64 (g1 = 0..S/2; col h -> freq _G2F[h])."""
    kx = trajectory[0].astype(np.float64)
    ky = trajectory[1].astype(np.float64)
    eta1 = kx * S / (2 * np.pi)
    eta2 = ky * S / (2 * np.pi)
    a0 = np.floor(eta1).astype(int) - J // 2 + 1
    b0 = np.floor(eta2).astype(int) - J // 2 + 1
    js = np.arange(J)
    w1 = _es_kernel(eta1[:, None] - (a0[:, None] + js[None, :]))  # [M, J]
    w2 = _es_kernel(eta2[:, None] - (b0[:, None] + js[None, :]))
    gg1 = (a0[:, None] + js[None, :]) % S                         # [M, J]
    gg2 = (b0[:, None] + js[None, :]) % S

    # full F grid [B, S, S] indexed by (g1 mod S, g2 mod S)
    Ffull = np.zeros((B, S, S), dtype=np.complex64)
    q2 = (_G2F % S)
    rows = np.arange(G1H)
    Ffull[:, rows[:, None] % S, q2[None, :]] = F
    neg = np.arange(1, S // 2)
    mirr = (S - np.arange(S)) % S
    Ffull[:, (-neg) % S, :] = np.conj(Ffull[:, neg][:, :, mirr])

    vals = Ffull[:, gg1[:, :, None], gg2[:, None, :]]             # [B, M, J, J]
    w = (w1[:, :, None] * w2[:, None, :]).astype(np.float32)      # [M, J, J]
    return (vals * w[None]).sum(axis=(2, 3)).astype(np.complex64)


def kernel(image, trajectory):
    from concourse.bass_utils import run_bass_kernel_spmd

    if 'nc' not in _CACHE:
        _CACHE['nc'] = _build()
    nc = _CACHE['nc']

    image = np.ascontiguousarray(np.asarray(image, dtype=np.float32))
    trajectory = np.ascontiguousarray(np.asarray(trajectory, dtype=np.float32))
    blobs = _host_prep(image, trajectory)

    in_maps = [{"blob": np.ascontiguousarray(blobs[c])} for c in range(NCORES)]
    res = run_bass_kernel_spmd(nc, in_maps, core_ids=list(range(NCORES)))

    F = np.zeros((B, G1H, S), dtype=np.complex64)
    for c in range(NCORES):
        b, q1, q2 = c // 4, (c // 2) % 2, c % 2
        o = res.results[c]["out"].astype(np.float32)   # [128, 2*SH]
        lo = q1 * RQ
        hi = min(lo + RQ, G1H)
        F[b, lo:hi, q2 * SH:(q2 + 1) * SH] = (o[0:hi - lo, 0:SH]
                                              + 1j * o[0:hi - lo, SH:2 * SH])

    return _host_interp(F, trajectory)
